# revision 25
# baseline (speedup 1.0000x reference)
"""Trainium2 Bass kernel for BaseBidirectionalAttention (BiDAF-style attention).

Reference computation (per batch b):
    sim[c,q]  = <w_c, ctx_c> + <w_q, q_q> + <w_m, ctx_c * q_q>
    c2q       = softmax_q(sim) @ question                      (C, E)
    q2c_w     = softmax_c(max_q sim)                           (C,)
    q2c       = q2c_w @ context                                (E,)
    attended  = [ctx, c2q, ctx*c2q, ctx*q2c]                   (C, 4E)
    out       = (attended @ final_W.T + final_b) * mask[:,None] (C, 4E)

Sharding: data-parallel over batch. 32 batches / 8 cores = 4 per core.
All parameters (final_W etc., <5MB) replicated on every core.

Device-side layouts (prepared host-side; pure layout transforms):
    ctxT   : context^T   (B, E, C)  fp32 (sim lhsT, block3/4 source)
    ctxT16 : context^T   bf16       (block1 of attended^T, final-matmul lhsT)
    ctxn   : context     (B, C, E)  fp32 (q2c contraction)
    qT     : question^T  (B, E, Q)  fp32 (sim rhs)
    q16    : question    bf16       (c2q lhsT)
    wt16   : final_W^T   (4E, 4E)   bf16 (final-matmul moving operand)

The whole attended^T (4E, C) is built on-chip in bf16, the final matmul runs
in bf16 (fp32 PSUM accumulate), bias is folded in via a K=1 matmul with a
ones row, and the context mask is applied by the ACT copy that evicts PSUM.
"""

import numpy as np
import ml_dtypes

import concourse.bass as bass
import concourse.mybir as mybir
import concourse.tile as tile
from concourse.bass_utils import run_bass_kernel_spmd
from concourse.masks import make_identity

B, C, Q, E = 32, 1024, 64, 256
FE = 4 * E
NCORES = 8
BL = B // NCORES  # batches per core

F32 = mybir.dt.float32
F32R = mybir.dt.float32r
BF16 = mybir.dt.bfloat16
AX = mybir.AxisListType.X
ALU = mybir.AluOpType
ACTF = mybir.ActivationFunctionType


def _split_multi_waits(nc):
    """The walrus build in this environment supports a single sync-wait per
    instruction. Move extra waits onto preceding same-engine NoOps."""
    counter = 0
    for f in nc.m.functions:
        for bb in f.blocks:
            insts = bb.instructions
            i = 0
            while i < len(insts):
                inst = insts[i]
                si = inst.sync_info
                waits = list(si.on_wait) if si is not None and si.on_wait else []
                if len(waits) > 1:
                    inst.sync_info = mybir.SyncInfo(
                        on_wait=[waits[-1]],
                        on_update=list(si.on_update) if si.on_update else [],
                    )
                    for w in waits[:-1]:
                        nop = mybir.InstNoOp(
                            name=f"I-swsplit-{counter}", engine=inst.engine
                        )
                        counter += 1
                        nop.sync_info = mybir.SyncInfo(on_wait=[w], on_update=[])
                        nc.register_instruction(nop)
                        insts.insert(i, nop)
                        i += 1
                i += 1


def _emit(nc, tc, dram, ctx, loop=1, bias_zero=False, mask_ones=False, probe=None):
    consts = ctx.enter_context(tc.tile_pool(name="consts", bufs=1))
    inp = ctx.enter_context(tc.tile_pool(name="inp", bufs=3))
    work = ctx.enter_context(tc.tile_pool(name="work", bufs=4))
    small = ctx.enter_context(tc.tile_pool(name="small", bufs=6))
    outp = ctx.enter_context(tc.tile_pool(name="outp", bufs=3))
    # PSUM: 8 banks total. Dedicated pools per phase so batch N+1's
    # attention never waits on batch N's final-matmul psums.
    # sim 2 + fin 2x2 + msc 2 = 8 banks (transposes share ps_msc).
    ps_sim = ctx.enter_context(tc.tile_pool(name="ps_sim", bufs=2, space="PSUM"))
    ps_fin = ctx.enter_context(tc.tile_pool(name="ps_fin", bufs=2, space="PSUM"))
    ps_msc = ctx.enter_context(tc.tile_pool(name="ps_msc", bufs=2, space="PSUM"))

    # ---- constants ----
    # (weights on the ACT HWDGE queue so the first batch's ctxT load on the
    # SP queue isn't stuck behind 2 MiB of weights)
    wt = consts.tile([128, 8, FE], BF16)  # final_W^T, k-chunk major
    nc.scalar.dma_start(wt[:], dram["wt16"].rearrange("(k p) f -> p k f", p=128))
    bvec = consts.tile([1, FE], BF16)
    nc.scalar.dma_start(bvec[:], dram["b16"][None, :])
    vecs = consts.tile([128, 2, 4], F32)  # cols: wq, wc, wm, 0 (e-chunked)
    nc.sync.dma_start(vecs[:], dram["vecs"].rearrange("(c p) v -> p c v", p=128))
    ones_b = consts.tile([1, 128], BF16)
    nc.vector.memset(ones_b[:], 1.0)
    ones_f = consts.tile([1, 128], F32)
    nc.vector.memset(ones_f[:], 1.0)
    ones_col = consts.tile([128, 1], F32)
    nc.vector.memset(ones_col[:], 1.0)
    ident = consts.tile([128, 128], F32)
    make_identity(nc, ident[:])
    ident16 = consts.tile([128, 128], BF16)
    nc.vector.tensor_copy(ident16[:], ident[:])
    if not bias_zero:
        b_b64 = consts.tile([64, FE], BF16)
        nc.sync.dma_start(
            b_b64[:],
            bass.AP(
                tensor=dram["b16"].tensor,
                offset=dram["b16"].offset,
                ap=[[0, 64]] + list(dram["b16"].ap),
            ),
        )

    if probe == "dma":
        dma_src = [
            consts.tile([128, FE], BF16, tag=f"dmasrc{i}", name=f"dmasrc{i}")
            for i in range(2)
        ]
        for t in dma_src:
            nc.gpsimd.memset(t[:], 1.0)

    def _attn_A(b):
        """Inputs + similarity matmuls + row softmax (no PE ops that wait on
        the softmax chain)."""
        # All inputs on the SP HWDGE queue (outputs own the ACT queue).
        # Smallest-and-earliest-needed first; ctxT split in halves so the
        # first 4 c-tiles' sim matmuls only wait on half 0.
        qT = inp.tile([128, 2, Q], F32, tag="qT")
        nc.sync.dma_start(qT[:], dram["qT"][b].rearrange("(c p) q -> p c q", p=128))
        q16 = inp.tile([64, E], BF16, tag="q16")
        nc.sync.dma_start(q16[:], dram["q16"][b])
        ctxT = inp.tile([128, 2, C], F32, tag="ctxT")
        ch = C // 2
        nc.sync.dma_start(
            ctxT[:, :, 0:ch],
            dram["ctxT"][b][:, 0:ch].rearrange("(c p) n -> p c n", p=128),
        )
        nc.sync.dma_start(
            ctxT[:, :, ch:C],
            dram["ctxT"][b][:, ch:C].rearrange("(c p) n -> p c n", p=128),
        )
        ctxn16 = inp.tile([128, 8, E], BF16, tag="ctxn16")
        nc.sync.dma_start(
            ctxn16[:], dram["ctxn16"][b].rearrange("(j p) e -> p j e", p=128)
        )
        if not mask_ones:
            mask_row = inp.tile([1, C], F32, tag="mask")
            nc.sync.dma_start(mask_row[:], dram["mask"][b : b + 1, :])

        # bf16 context^T (final-matmul block 1), derived on-device
        ctxT16 = work.tile([128, 2, C], BF16, tag="ctxT16")
        for ec in range(2):
            nc.vector.tensor_copy(ctxT16[:, ec, :], ctxT[:, ec, :])

        if probe == "dma":
            for cs in range(8):
                nc.sync.dma_start(
                    dram["out"][b, cs * 128 : (cs + 1) * 128, :], dma_src[cs % 2][:]
                )
            return None

        if probe == "mmonly":
            # dense self-loading bf16 matmuls, minimal eviction: HW ns/MM
            for cs in range(8):
                csl = slice(cs * 128, (cs + 1) * 128)
                for fh in range(2):
                    fhl = slice(fh * 512, (fh + 1) * 512)
                    ps_o = ps_fin.tile([128, 512], F32, tag="fin")
                    for kc in range(8):
                        nc.tensor.matmul(
                            ps_o[:], ctxT16[:, kc % 2, csl], wt[:, kc, fhl],
                            start=(kc == 0), stop=(kc == 7),
                        )
                    s = small.tile([128, 1], F32, tag="mmsink")
                    nc.vector.tensor_copy(s[:], ps_o[:, 0:1])
            return None

        if probe == "mmpair":
            # like mmonly but each stationary serves 2 matmuls (fh pair)
            for cs in range(8):
                csl = slice(cs * 128, (cs + 1) * 128)
                ps_o = ps_fin.tile([128, 2, 512], F32, tag="fin")
                for kc in range(8):
                    for fh in range(2):
                        fhl = slice(fh * 512, (fh + 1) * 512)
                        nc.tensor.matmul(
                            ps_o[:, fh, :], ctxT16[:, kc % 2, csl], wt[:, kc, fhl],
                            start=(kc == 0), stop=(kc == 7),
                        )
                s = small.tile([128, 1], F32, tag="mmsink")
                nc.vector.tensor_copy(s[:], ps_o[:, 0, 0:1])
            return None

        if probe == "mm256":
            for cs in range(8):
                csl = slice(cs * 128, (cs + 1) * 128)
                for fh in range(4):
                    fhl = slice(fh * 256, (fh + 1) * 256)
                    ps_o = ps_fin.tile([128, 256], F32, tag="fin")
                    for kc in range(8):
                        nc.tensor.matmul(
                            ps_o[:], ctxT16[:, kc % 2, csl], wt[:, kc, fhl],
                            start=(kc == 0), stop=(kc == 7),
                        )
                    s = small.tile([128, 1], F32, tag="mmsink")
                    nc.vector.tensor_copy(s[:], ps_o[:, 0:1])
            return None

        if probe == "final":
            att2 = att3 = att4 = ctxT16
            for cs in range(8):
                csl = slice(cs * 128, (cs + 1) * 128)
                out_s = outp.tile([128, FE], F32, tag="out")
                for fh in range(2):
                    fhl = slice(fh * 512, (fh + 1) * 512)
                    ps_o = ps_fin.tile([128, 512], F32, tag="fin")
                    for kc in range(8):
                        blk = (ctxT16, att2, att3, att4)[kc // 2]
                        nc.tensor.matmul(
                            ps_o[:], blk[:, kc % 2, csl], wt[:, kc, fhl],
                            start=(kc == 0), stop=(kc == 7),
                        )
                    nc.scalar.copy(out_s[:, fhl], ps_o[:])
                nc.sync.dma_start(dram["out"][b, csl, :], out_s[:])
            return None

        # ---- rhs_ext = [qT * w_multiple | w_context]; col Q -> ctxw ----
        rhs_ext = work.tile([128, 2, Q + 1], F32, tag="qTs")
        for ec in range(2):
            nc.vector.tensor_scalar_mul(
                rhs_ext[:, ec, 0:Q], qT[:, ec, :], vecs[:, ec, 2:3]
            )
            nc.vector.tensor_copy(rhs_ext[:, ec, Q : Q + 1], vecs[:, ec, 1:2])

        # ---- q_weighted row: qw[q] = <w_question, question_q> ----
        # kept as a bf16 hi+lo pair so the per-c-tile broadcast matmuls run
        # at bf16 rate (1 cyc/col) instead of fp32 (4 cyc/col)
        ps_qw = ps_msc.tile([1, Q], F32, tag="msc")
        for ec in range(2):
            nc.tensor.matmul(
                ps_qw[:], vecs[:, ec, 0:1], qT[:, ec, :],
                start=(ec == 0), stop=(ec == 1),
            )
        qw_f = work.tile([1, Q + 1], F32, tag="qwf")  # [qw | 0] fp32
        nc.vector.memset(qw_f[:], 0.0)
        nc.scalar.copy(qw_f[0:1, 0:Q], ps_qw[:])
        qw_hi = work.tile([1, Q + 1], BF16, tag="qwhi")
        nc.vector.tensor_copy(qw_hi[:], qw_f[:])
        qw_res = work.tile([1, Q + 1], F32, tag="qwres")
        nc.vector.tensor_sub(qw_res[:], qw_f[:], qw_hi[:])
        qw_lo = work.tile([1, Q + 1], BF16, tag="qwlo")
        nc.vector.tensor_copy(qw_lo[:], qw_res[:])

        # ---- mask columns ----
        mask_c = None
        if not mask_ones:
            mask_c = work.tile([128, 8], F32, tag="mask_c")
            for cs in range(8):
                ps_mc = ps_msc.tile([128, 1], F32, tag="msc")
                nc.tensor.matmul(
                    ps_mc[:],
                    mask_row[0:1, cs * 128 : (cs + 1) * 128],
                    ones_f[0:1, 0:1],
                    start=True,
                    stop=True,
                )
                nc.scalar.copy(mask_c[:, cs : cs + 1], ps_mc[:])

        # ---- similarity: logits (mult + qw[q]) cols 0..Q, ctxw[c] in col Q.
        negrow = work.tile([128, 8], F32, tag="negrow")
        ctxw_c = work.tile([128, 8], F32, tag="ctxw_c")
        p = work.tile([128, 8, Q], BF16, tag="p")
        zrows = work.tile([128, 8], F32, tag="zrows")
        rz = work.tile([128, 8], F32, tag="rz")
        for h in range(2):
            ps_s = ps_sim.tile([128, 4, Q + 1], F32, tag="sim")
            for k in range(4):
                cs = h * 4 + k
                csl = slice(cs * 128, (cs + 1) * 128)
                nc.tensor.matmul(
                    ps_s[:, k, :], ctxT[:, 0, csl], rhs_ext[:, 0, :],
                    start=(k == 0), stop=False,
                )
                nc.tensor.matmul(
                    ps_s[:, k, :], ctxT[:, 1, csl], rhs_ext[:, 1, :],
                    start=False, stop=False,
                )
            # qw broadcast rows last: the ones_b stationary is loaded once
            # for all 8 of these matmuls instead of being re-loaded between
            # ctx matmuls
            for k in range(4):
                nc.tensor.matmul(
                    ps_s[:, k, :], ones_b[:], qw_hi[:], start=False, stop=False
                )
            for k in range(4):
                nc.tensor.matmul(
                    ps_s[:, k, :], ones_b[:], qw_lo[:], start=False, stop=(k == 3)
                )
            hl = slice(h * 4, (h + 1) * 4)
            nc.vector.reduce_max(
                out=negrow[:, hl], in_=ps_s[:, :, 0:Q], axis=AX, negate=True
            )
            nc.scalar.copy(
                ctxw_c[:, hl], ps_s[:, :, Q : Q + 1].rearrange("p a b -> p (a b)")
            )
            for k in range(4):
                cs = h * 4 + k
                nc.scalar.activation(
                    out=p[:, cs, :],
                    in_=ps_s[:, k, 0:Q],
                    func=ACTF.Exp,
                    bias=negrow[:, cs : cs + 1],
                    scale=1.0,
                    accum_out=zrows[:, cs : cs + 1],
                )
        nc.vector.reciprocal(rz[:], zrows[:])
        for cs in range(8):
            nc.vector.tensor_scalar_mul(p[:, cs, :], p[:, cs, :], rz[:, cs : cs + 1])

        # ---- qW2 = question @ W2^T (+ bias: softmax rows sum to 1, so
        # adding b here adds exactly b to the output) ----
        qT16 = work.tile([128, 2, Q], BF16, tag="qT16")
        for ec in range(2):
            nc.vector.tensor_copy(qT16[:, ec, :], qT[:, ec, :])
        qw2 = work.tile([64, FE], BF16, tag="qw2")
        # ec-outer so each qT16 stationary is loaded once for both fh halves
        ps_qs = [
            ps_msc.tile([64, 512], F32, tag="msc", name=f"ps_q{fh}")
            for fh in range(2)
        ]
        for ec in range(2):
            for fh in range(2):
                fhl = slice(fh * 512, (fh + 1) * 512)
                nc.tensor.matmul(
                    ps_qs[fh][:], qT16[:, ec, :], wt[:, ec + 2, fhl],
                    start=(ec == 0), stop=(ec == 1),
                )
        for fh in range(2):
            fhl = slice(fh * 512, (fh + 1) * 512)
            if bias_zero:
                nc.scalar.copy(qw2[:, fhl], ps_qs[fh][:])
            else:
                nc.vector.tensor_add(qw2[:, fhl], ps_qs[fh][:], b_b64[:, fhl])

        return dict(
            b=b, ctxT=ctxT, ctxT16=ctxT16, q16=q16, ctxn16=ctxn16,
            mask_c=mask_c, negrow=negrow, ctxw_c=ctxw_c, p=p, qw2=qw2,
        )

    def _attn_B(st):
        """PE ops that wait on the softmax chain: transposes, c2q, q2c."""
        ctxT, ctxT16, q16, ctxn16 = st["ctxT"], st["ctxT16"], st["q16"], st["ctxn16"]
        negrow, ctxw_c, p = st["negrow"], st["ctxw_c"], st["p"]

        # ---- P^T via PE transposes (4 chunks per psum tile) ----
        pnt = work.tile([64, C], BF16, tag="pnt")  # P_norm^T
        for h in range(2):
            ps_t = ps_msc.tile([64, 4, 128], BF16, tag="msc")
            for k in range(4):
                cs = h * 4 + k
                nc.tensor.transpose(ps_t[:, k, :], p[:, cs, :], ident16[:])
            nc.vector.tensor_copy(
                pnt[:, h * 512 : (h + 1) * 512],
                ps_t[:].rearrange("p a b -> p (a b)"),
            )

        # ---- c2q attention (only needed for block3 = ctx * c2q) ----
        att3 = work.tile([128, 2, C], BF16, tag="att3")  # (ctx*c2q)^T
        for ec in range(2):
            for ch in range(2):
                chl = slice(ch * 512, (ch + 1) * 512)
                ps_c2q = ps_msc.tile([128, 512], F32, tag="msc")
                nc.tensor.matmul(
                    ps_c2q[:],
                    q16[:, ec * 128 : (ec + 1) * 128],
                    pnt[:, chl],
                    start=True,
                    stop=True,
                )
                nc.vector.tensor_mul(att3[:, ec, chl], ctxT[:, ec, chl], ps_c2q[:])

        # ---- q2c attention + attended block 4 (feeds wtm, needed late) ----
        rowtrue = work.tile([128, 8], F32, tag="rowtrue")
        nc.vector.tensor_sub(rowtrue[:], ctxw_c[:], negrow[:])
        colmax = small.tile([128, 1], F32, tag="colmax")
        nc.vector.reduce_max(out=colmax[:], in_=rowtrue[:], axis=AX)
        ps_t1 = ps_msc.tile([1, 128], F32, tag="msc")
        nc.tensor.transpose(ps_t1[:], colmax[:], ident[:])
        tmax = small.tile([1, 128], F32, tag="tmax")
        nc.scalar.copy(tmax[:], ps_t1[:])
        gneg = small.tile([1, 1], F32, tag="gneg")  # -gmax
        nc.vector.reduce_max(out=gneg[:], in_=tmax[:], axis=AX, negate=True)
        ps_gb = ps_msc.tile([128, 1], F32, tag="msc")
        nc.tensor.matmul(ps_gb[:], ones_f[:], gneg[:], start=True, stop=True)
        gneg_col = small.tile([128, 1], F32, tag="gnegc")
        nc.scalar.copy(gneg_col[:], ps_gb[:])
        e_t = work.tile([128, 8], F32, tag="e_t")  # exp(rowtrue - gmax)
        zrow2 = small.tile([128, 1], F32, tag="zrow2")
        nc.scalar.activation(
            out=e_t[:],
            in_=rowtrue[:],
            func=ACTF.Exp,
            bias=gneg_col[:],
            scale=1.0,
            accum_out=zrow2[:],
        )
        e16 = work.tile([128, 8], BF16, tag="e16")
        nc.vector.tensor_copy(e16[:], e_t[:])
        ps_z = ps_msc.tile([1, 1], F32, tag="msc")
        nc.tensor.matmul(ps_z[:], zrow2[:], ones_col[:], start=True, stop=True)
        z_s = small.tile([1, 1], F32, tag="z_s")
        nc.scalar.copy(z_s[:], ps_z[:])
        rz1 = small.tile([1, 1], F32, tag="rz1")
        nc.vector.reciprocal(rz1[:], z_s[:])
        ps_q2c = ps_msc.tile([1, E], F32, tag="msc")
        for j in range(8):
            nc.tensor.matmul(
                ps_q2c[:],
                e16[:, j : j + 1],
                ctxn16[:, j, :],
                start=(j == 0),
                stop=(j == 7),
            )
        # 1/Z folded into the eviction copy
        q2c_row = small.tile([1, E], F32, tag="q2c_row")
        nc.scalar.activation(
            out=q2c_row[:], in_=ps_q2c[:], func=ACTF.Copy, scale=rz1[:]
        )
        # block4 never materializes: (ctx . q2c) @ W4^T == ctx @ (q2c . W4^T),
        # so fold q2c into merged weights for the ctx block instead.
        wtm = work.tile([128, 2, FE], BF16, tag="wtm")  # W1^T + q2c . W4^T
        for ec in range(2):
            ps_qc = ps_msc.tile([128, 1], F32, tag="msc")
            nc.tensor.matmul(
                ps_qc[:],
                q2c_row[0:1, ec * 128 : (ec + 1) * 128],
                ones_f[0:1, 0:1],
                start=True,
                stop=True,
            )
            q2c_col = small.tile([128, 1], F32, tag="q2c_col")
            nc.scalar.copy(q2c_col[:], ps_qc[:])
            nc.vector.tensor_scalar_mul(wtm[:, ec, :], wt[:, ec + 6, :], q2c_col[:])
            nc.vector.tensor_add(wtm[:, ec, :], wtm[:, ec, :], wt[:, ec, :])

        st["att3"], st["wtm"], st["pnt"] = att3, wtm, pnt

        if probe == "attn":
            b = st["b"]
            sink = outp.tile([128, 32], BF16, tag="sink")
            nc.vector.tensor_copy(sink[:, 0:8], st["qw2"][0:64, 0:8])
            nc.vector.tensor_copy(sink[:, 8:16], att3[:, 0, 0:8])
            nc.vector.tensor_copy(sink[:, 16:24], wtm[:, 0, 0:8])
            nc.vector.tensor_copy(sink[:, 24:32], ctxT16[:, 0, 0:8])
            nc.sync.dma_start(dram["out"][b, 0:128, 0:32], sink[:])
            st["skip_final"] = True

    def _final_half(st, half):
        # ---- final matmul: out = (attended @ W^T + b) * mask ----
        if st.get("skip_final"):
            return
        b, ctxT16 = st["b"], st["ctxT16"]
        att3, wtm, mask_c = st["att3"], st["wtm"], st["mask_c"]
        pnt, qw2 = st["pnt"], st["qw2"]
        for cs in range(half * 4, half * 4 + 4):
            csl = slice(cs * 128, (cs + 1) * 128)
            out_s = outp.tile([128, FE], BF16, tag="out")
            # one [128, 2, 512] psum pair (2 banks): each stationary operand
            # (attended^T chunk) is loaded once and used for both output
            # halves, halving LDWEIGHTS traffic.
            # merged ctx block (K=256), P-hat block (K=64), ctx*c2q (K=256)
            ps_o = ps_fin.tile([128, 2, 512], F32, tag="fin")
            blocks = [
                (pnt[:, csl], qw2[:, :]),
                (att3[:, 0, csl], wt[:, 4, :]),
                (att3[:, 1, csl], wt[:, 5, :]),
                (ctxT16[:, 0, csl], wtm[:, 0, :]),
                (ctxT16[:, 1, csl], wtm[:, 1, :]),
            ]
            nblk = len(blocks)
            for i, (lhsT, rhs) in enumerate(blocks):
                for fh in range(2):
                    fhl = slice(fh * 512, (fh + 1) * 512)
                    nc.tensor.matmul(
                        ps_o[:, fh, :], lhsT, rhs[:, fhl],
                        start=(i == 0), stop=(i == nblk - 1),
                    )
            for fh in range(2):
                fhl = slice(fh * 512, (fh + 1) * 512)
                if mask_ones:
                    # alternate evictions between ACT and DVE
                    if fh == 0:
                        nc.scalar.copy(out_s[:, fhl], ps_o[:, fh, :])
                    else:
                        nc.vector.tensor_copy(out_s[:, fhl], ps_o[:, fh, :])
                else:
                    nc.scalar.activation(
                        out=out_s[:, fhl],
                        in_=ps_o[:, fh, :],
                        func=ACTF.Copy,
                        scale=mask_c[:, cs : cs + 1],
                    )
            # one merged output DMA per c-tile, on the ACT HWDGE queue (the
            # inputs use the SP queue; splitting halves descriptor-gen serial
            # time)
            nc.scalar.dma_start(dram["out"][b, csl, :], out_s[:])

    def _all_batches(prev=None):
        # software pipeline with the final matmul split in halves around the
        # softmax-dependent PE ops of the NEXT batch, so the in-order PE
        # stream never waits on the softmax chain:
        #   simMMs(b+1) | final(b).half0 | transposes/c2q/q2c(b+1) |
        #   final(b).half1
        # `prev` carries the pipeline across unrolled loop bodies.
        for b in range(BL):
            st = _attn_A(b)
            if st is None:
                continue
            if prev is None:
                _attn_B(st)
            else:
                _final_half(prev, 0)
                _attn_B(st)
                _final_half(prev, 1)
            prev = st
        return prev

    def _finish(prev):
        if prev is not None:
            _final_half(prev, 0)
            _final_half(prev, 1)

    if loop > 1:
        # unroll the hardware loop: For_i ends each iteration with an
        # all-engine barrier (semaphore reset), so emitting U bodies per
        # iteration amortizes the pipeline drain/ramp 1/U
        U = 8 if loop % 8 == 0 else (4 if loop % 4 == 0 else 1)
        with tc.For_i(
            0,
            loop // U,
            1,
            hint_engines=(
                mybir.EngineType.PE,
                mybir.EngineType.DVE,
                mybir.EngineType.Activation,
                mybir.EngineType.SP,
                mybir.EngineType.Pool,
            ),
        ):
            prev = None
            for _ in range(U):
                prev = _all_batches(prev)
            _finish(prev)
    else:
        _finish(_all_batches())
    if "stub" in dram:
        nc.sync.dma_start(dram["stub"][:], ones_f[0:1, 0:8])


_NC_CACHE = {}


def _get_nc(loop=1, bias_zero=False, mask_ones=False, probe=None):
    key = (loop, bias_zero, mask_ones, probe)
    if key not in _NC_CACHE:
        nc = bass.Bass("TRN2", target_bir_lowering=False, debug=False,
                       num_devices=NCORES)
        dram = {
            "ctxT": nc.dram_tensor("ctxT", [BL, E, C], F32, kind="ExternalInput").ap(),
            "ctxn16": nc.dram_tensor(
                "ctxn16", [BL, C, E], BF16, kind="ExternalInput"
            ).ap(),
            "qT": nc.dram_tensor("qT", [BL, E, Q], F32, kind="ExternalInput").ap(),
            "q16": nc.dram_tensor("q16", [BL, Q, E], BF16, kind="ExternalInput").ap(),
            "mask": nc.dram_tensor("mask", [BL, C], F32, kind="ExternalInput").ap(),
            "wt16": nc.dram_tensor("wt16", [FE, FE], BF16, kind="ExternalInput").ap(),
            "b16": nc.dram_tensor("b16", [FE], BF16, kind="ExternalInput").ap(),
            "vecs": nc.dram_tensor("vecs", [E, 4], F32, kind="ExternalInput").ap(),
        }
        if loop > 1:
            # timing variant: keep the big output on-device, return a stub
            dram["out"] = nc.dram_tensor("out_int", [BL, C, FE], BF16).ap()
            dram["stub"] = nc.dram_tensor(
                "out", [1, 8], F32, kind="ExternalOutput"
            ).ap()
        else:
            dram["out"] = nc.dram_tensor(
                "out", [BL, C, FE], BF16, kind="ExternalOutput"
            ).ap()
        from contextlib import ExitStack

        with tile.TileContext(nc) as tc, ExitStack() as es:
            _emit(nc, tc, dram, es, loop=loop, bias_zero=bias_zero,
                  mask_ones=mask_ones, probe=probe)
        _split_multi_waits(nc)
        _NC_CACHE[key] = nc
    return _NC_CACHE[key]


def _prep_inputs(context, question, context_mask, w_question, w_context, w_multiple,
                 final_W, final_b):
    """Host-side layout prep + sharding. Returns per-core input maps."""
    bf16 = ml_dtypes.bfloat16
    context = np.asarray(context, np.float32)
    question = np.asarray(question, np.float32)
    ctxT = np.ascontiguousarray(context.transpose(0, 2, 1))
    ctx16 = context.astype(bf16)
    qT = np.ascontiguousarray(question.transpose(0, 2, 1))
    q16 = question.astype(bf16)
    wt16 = np.ascontiguousarray(np.asarray(final_W, np.float32).T).astype(bf16)
    b16 = np.asarray(final_b, np.float32).astype(bf16)
    vecs = np.stack(
        [
            np.asarray(w_question, np.float32),
            np.asarray(w_context, np.float32),
            np.asarray(w_multiple, np.float32),
            np.zeros(E, np.float32),
        ],
        axis=1,
    )
    mask = np.asarray(context_mask, np.float32)
    in_maps = []
    for i in range(NCORES):
        s = slice(i * BL, (i + 1) * BL)
        in_maps.append(
            {
                "ctxT": ctxT[s],
                "ctxn16": ctx16[s],
                "qT": qT[s],
                "q16": q16[s],
                "mask": mask[s],
                "wt16": wt16,
                "b16": b16,
                "vecs": vecs,
            }
        )
    return in_maps


def kernel(context, question, context_mask, w_question, w_context, w_multiple,
           final_W, final_b, _loop=1, _probe=None, **run_kwargs):
    bias_zero = not np.any(np.asarray(final_b))
    mask_ones = bool(np.all(np.asarray(context_mask) == 1.0))
    nc = _get_nc(loop=_loop, bias_zero=bias_zero, mask_ones=mask_ones, probe=_probe)
    in_maps = _prep_inputs(
        context, question, context_mask, w_question, w_context, w_multiple,
        final_W, final_b,
    )
    res = run_bass_kernel_spmd(nc, in_maps, core_ids=list(range(NCORES)), **run_kwargs)
    if _loop > 1:
        return res
    out = np.empty((B, C, FE), np.float32)
    for i in range(NCORES):
        out[i * BL : (i + 1) * BL] = res.results[i]["out"].astype(np.float32)
    if run_kwargs:
        kernel.last_results = res
    return out



# revision 27
# speedup vs baseline: 1.0278x; 1.0278x over previous
"""Trainium2 Bass kernel for BaseBidirectionalAttention (BiDAF-style attention).

Reference computation (per batch b):
    sim[c,q]  = <w_c, ctx_c> + <w_q, q_q> + <w_m, ctx_c * q_q>
    c2q       = softmax_q(sim) @ question                      (C, E)
    q2c_w     = softmax_c(max_q sim)                           (C,)
    q2c       = q2c_w @ context                                (E,)
    attended  = [ctx, c2q, ctx*c2q, ctx*q2c]                   (C, 4E)
    out       = (attended @ final_W.T + final_b) * mask[:,None] (C, 4E)

Sharding: data-parallel over batch. 32 batches / 8 cores = 4 per core.
All parameters (final_W etc., <5MB) replicated on every core.

Device-side layouts (prepared host-side; pure layout transforms):
    ctxT   : context^T   (B, E, C)  fp32 (sim lhsT, block3/4 source)
    ctxT16 : context^T   bf16       (block1 of attended^T, final-matmul lhsT)
    ctxn   : context     (B, C, E)  fp32 (q2c contraction)
    qT     : question^T  (B, E, Q)  fp32 (sim rhs)
    q16    : question    bf16       (c2q lhsT)
    wt16   : final_W^T   (4E, 4E)   bf16 (final-matmul moving operand)

The whole attended^T (4E, C) is built on-chip in bf16, the final matmul runs
in bf16 (fp32 PSUM accumulate), bias is folded in via a K=1 matmul with a
ones row, and the context mask is applied by the ACT copy that evicts PSUM.
"""

import numpy as np
import ml_dtypes

import concourse.bass as bass
import concourse.mybir as mybir
import concourse.tile as tile
from concourse.bass_utils import run_bass_kernel_spmd
from concourse.masks import make_identity

B, C, Q, E = 32, 1024, 64, 256
FE = 4 * E
NCORES = 8
BL = B // NCORES  # batches per core

F32 = mybir.dt.float32
F32R = mybir.dt.float32r
BF16 = mybir.dt.bfloat16
AX = mybir.AxisListType.X
ALU = mybir.AluOpType
ACTF = mybir.ActivationFunctionType


def _split_multi_waits(nc):
    """The walrus build in this environment supports a single sync-wait per
    instruction. Move extra waits onto preceding same-engine NoOps."""
    counter = 0
    for f in nc.m.functions:
        for bb in f.blocks:
            insts = bb.instructions
            i = 0
            while i < len(insts):
                inst = insts[i]
                si = inst.sync_info
                waits = list(si.on_wait) if si is not None and si.on_wait else []
                if len(waits) > 1:
                    inst.sync_info = mybir.SyncInfo(
                        on_wait=[waits[-1]],
                        on_update=list(si.on_update) if si.on_update else [],
                    )
                    for w in waits[:-1]:
                        nop = mybir.InstNoOp(
                            name=f"I-swsplit-{counter}", engine=inst.engine
                        )
                        counter += 1
                        nop.sync_info = mybir.SyncInfo(on_wait=[w], on_update=[])
                        nc.register_instruction(nop)
                        insts.insert(i, nop)
                        i += 1
                i += 1


def _emit(nc, tc, dram, ctx, loop=1, bias_zero=False, mask_ones=False, probe=None):
    consts = ctx.enter_context(tc.tile_pool(name="consts", bufs=1))
    inp = ctx.enter_context(tc.tile_pool(name="inp", bufs=3))
    work = ctx.enter_context(tc.tile_pool(name="work", bufs=4))
    small = ctx.enter_context(tc.tile_pool(name="small", bufs=6))
    outp = ctx.enter_context(tc.tile_pool(name="outp", bufs=3))
    # PSUM: 8 banks total. Dedicated pools per phase so batch N+1's
    # attention never waits on batch N's final-matmul psums.
    # sim 2 + fin 2x2 + msc 2 = 8 banks (transposes share ps_msc).
    ps_sim = ctx.enter_context(tc.tile_pool(name="ps_sim", bufs=2, space="PSUM"))
    ps_fin = ctx.enter_context(tc.tile_pool(name="ps_fin", bufs=2, space="PSUM"))
    ps_msc = ctx.enter_context(tc.tile_pool(name="ps_msc", bufs=2, space="PSUM"))

    # ---- constants ----
    # (weights on the ACT HWDGE queue so the first batch's ctxT load on the
    # SP queue isn't stuck behind 2 MiB of weights)
    wt = consts.tile([128, 8, FE], BF16)  # final_W^T, k-chunk major
    nc.scalar.dma_start(wt[:], dram["wt16"].rearrange("(k p) f -> p k f", p=128))
    bvec = consts.tile([1, FE], BF16)
    nc.scalar.dma_start(bvec[:], dram["b16"][None, :])
    vecs = consts.tile([128, 2, 4], F32)  # cols: wq, wc, wm, 0 (e-chunked)
    nc.sync.dma_start(vecs[:], dram["vecs"].rearrange("(c p) v -> p c v", p=128))
    ones_b = consts.tile([1, 128], BF16)
    nc.vector.memset(ones_b[:], 1.0)
    ones_f = consts.tile([1, 128], F32)
    nc.vector.memset(ones_f[:], 1.0)
    ones_col = consts.tile([128, 1], F32)
    nc.vector.memset(ones_col[:], 1.0)
    ident = consts.tile([128, 128], F32)
    make_identity(nc, ident[:])
    ident16 = consts.tile([128, 128], BF16)
    nc.vector.tensor_copy(ident16[:], ident[:])
    if not bias_zero:
        b_b64 = consts.tile([64, FE], BF16)
        nc.sync.dma_start(
            b_b64[:],
            bass.AP(
                tensor=dram["b16"].tensor,
                offset=dram["b16"].offset,
                ap=[[0, 64]] + list(dram["b16"].ap),
            ),
        )

    if probe == "dma":
        dma_src = [
            consts.tile([128, FE], BF16, tag=f"dmasrc{i}", name=f"dmasrc{i}")
            for i in range(2)
        ]
        for t in dma_src:
            nc.gpsimd.memset(t[:], 1.0)

    def _attn_A(b):
        """Inputs + similarity matmuls + row softmax (no PE ops that wait on
        the softmax chain)."""
        # All inputs on the SP HWDGE queue (outputs own the ACT queue).
        # Smallest-and-earliest-needed first; ctxT split in halves so the
        # first 4 c-tiles' sim matmuls only wait on half 0.
        qT = inp.tile([128, 2, Q], F32, tag="qT")
        nc.sync.dma_start(qT[:], dram["qT"][b].rearrange("(c p) q -> p c q", p=128))
        q16 = inp.tile([64, E], BF16, tag="q16")
        nc.sync.dma_start(q16[:], dram["q16"][b])
        ctxT = inp.tile([128, 2, C], F32, tag="ctxT")
        ch = C // 2
        nc.sync.dma_start(
            ctxT[:, :, 0:ch],
            dram["ctxT"][b][:, 0:ch].rearrange("(c p) n -> p c n", p=128),
        )
        nc.sync.dma_start(
            ctxT[:, :, ch:C],
            dram["ctxT"][b][:, ch:C].rearrange("(c p) n -> p c n", p=128),
        )
        ctxn16 = inp.tile([128, 8, E], BF16, tag="ctxn16")
        nc.sync.dma_start(
            ctxn16[:], dram["ctxn16"][b].rearrange("(j p) e -> p j e", p=128)
        )
        if not mask_ones:
            mask_row = inp.tile([1, C], F32, tag="mask")
            nc.sync.dma_start(mask_row[:], dram["mask"][b : b + 1, :])

        # bf16 context^T (final-matmul block 1), derived on-device
        ctxT16 = work.tile([128, 2, C], BF16, tag="ctxT16")
        for ec in range(2):
            nc.vector.tensor_copy(ctxT16[:, ec, :], ctxT[:, ec, :])

        if probe == "dma":
            for cs in range(8):
                nc.sync.dma_start(
                    dram["out"][b, cs * 128 : (cs + 1) * 128, :], dma_src[cs % 2][:]
                )
            return None

        if probe == "mmonly":
            # dense self-loading bf16 matmuls, minimal eviction: HW ns/MM
            for cs in range(8):
                csl = slice(cs * 128, (cs + 1) * 128)
                for fh in range(2):
                    fhl = slice(fh * 512, (fh + 1) * 512)
                    ps_o = ps_fin.tile([128, 512], F32, tag="fin")
                    for kc in range(8):
                        nc.tensor.matmul(
                            ps_o[:], ctxT16[:, kc % 2, csl], wt[:, kc, fhl],
                            start=(kc == 0), stop=(kc == 7),
                        )
                    s = small.tile([128, 1], F32, tag="mmsink")
                    nc.vector.tensor_copy(s[:], ps_o[:, 0:1])
            return None

        if probe == "mmpair":
            # like mmonly but each stationary serves 2 matmuls (fh pair)
            for cs in range(8):
                csl = slice(cs * 128, (cs + 1) * 128)
                ps_o = ps_fin.tile([128, 2, 512], F32, tag="fin")
                for kc in range(8):
                    for fh in range(2):
                        fhl = slice(fh * 512, (fh + 1) * 512)
                        nc.tensor.matmul(
                            ps_o[:, fh, :], ctxT16[:, kc % 2, csl], wt[:, kc, fhl],
                            start=(kc == 0), stop=(kc == 7),
                        )
                s = small.tile([128, 1], F32, tag="mmsink")
                nc.vector.tensor_copy(s[:], ps_o[:, 0, 0:1])
            return None

        if probe == "mm256":
            for cs in range(8):
                csl = slice(cs * 128, (cs + 1) * 128)
                for fh in range(4):
                    fhl = slice(fh * 256, (fh + 1) * 256)
                    ps_o = ps_fin.tile([128, 256], F32, tag="fin")
                    for kc in range(8):
                        nc.tensor.matmul(
                            ps_o[:], ctxT16[:, kc % 2, csl], wt[:, kc, fhl],
                            start=(kc == 0), stop=(kc == 7),
                        )
                    s = small.tile([128, 1], F32, tag="mmsink")
                    nc.vector.tensor_copy(s[:], ps_o[:, 0:1])
            return None

        if probe == "final":
            att2 = att3 = att4 = ctxT16
            for cs in range(8):
                csl = slice(cs * 128, (cs + 1) * 128)
                out_s = outp.tile([128, FE], F32, tag="out")
                for fh in range(2):
                    fhl = slice(fh * 512, (fh + 1) * 512)
                    ps_o = ps_fin.tile([128, 512], F32, tag="fin")
                    for kc in range(8):
                        blk = (ctxT16, att2, att3, att4)[kc // 2]
                        nc.tensor.matmul(
                            ps_o[:], blk[:, kc % 2, csl], wt[:, kc, fhl],
                            start=(kc == 0), stop=(kc == 7),
                        )
                    nc.scalar.copy(out_s[:, fhl], ps_o[:])
                nc.sync.dma_start(dram["out"][b, csl, :], out_s[:])
            return None

        # ---- rhs_ext = [qT * w_multiple | w_context]; col Q -> ctxw ----
        rhs_ext = work.tile([128, 2, Q + 1], F32, tag="qTs")
        for ec in range(2):
            nc.vector.tensor_scalar_mul(
                rhs_ext[:, ec, 0:Q], qT[:, ec, :], vecs[:, ec, 2:3]
            )
            nc.vector.tensor_copy(rhs_ext[:, ec, Q : Q + 1], vecs[:, ec, 1:2])

        # ---- q_weighted row: qw[q] = <w_question, question_q> ----
        # kept as a bf16 hi+lo pair so the per-c-tile broadcast matmuls run
        # at bf16 rate (1 cyc/col) instead of fp32 (4 cyc/col)
        ps_qw = ps_msc.tile([1, Q], F32, tag="msc")
        for ec in range(2):
            nc.tensor.matmul(
                ps_qw[:], vecs[:, ec, 0:1], qT[:, ec, :],
                start=(ec == 0), stop=(ec == 1),
            )
        qw_f = work.tile([1, Q + 1], F32, tag="qwf")  # [qw | 0] fp32
        nc.vector.memset(qw_f[:], 0.0)
        nc.scalar.copy(qw_f[0:1, 0:Q], ps_qw[:])
        qw_hi = work.tile([1, Q + 1], BF16, tag="qwhi")
        nc.vector.tensor_copy(qw_hi[:], qw_f[:])
        qw_res = work.tile([1, Q + 1], F32, tag="qwres")
        nc.vector.tensor_sub(qw_res[:], qw_f[:], qw_hi[:])
        qw_lo = work.tile([1, Q + 1], BF16, tag="qwlo")
        nc.vector.tensor_copy(qw_lo[:], qw_res[:])

        # ---- mask columns ----
        mask_c = None
        if not mask_ones:
            mask_c = work.tile([128, 8], F32, tag="mask_c")
            for cs in range(8):
                ps_mc = ps_msc.tile([128, 1], F32, tag="msc")
                nc.tensor.matmul(
                    ps_mc[:],
                    mask_row[0:1, cs * 128 : (cs + 1) * 128],
                    ones_f[0:1, 0:1],
                    start=True,
                    stop=True,
                )
                nc.scalar.copy(mask_c[:, cs : cs + 1], ps_mc[:])

        # ---- similarity: logits (mult + qw[q]) cols 0..Q, ctxw[c] in col Q.
        negrow = work.tile([128, 8], F32, tag="negrow")
        ctxw_c = work.tile([128, 8], F32, tag="ctxw_c")
        p = work.tile([128, 8, Q], BF16, tag="p")
        praw = work.tile([128, 8, Q], BF16, tag="praw")
        zrows = work.tile([128, 8], F32, tag="zrows")
        rz = work.tile([128, 8], F32, tag="rz")
        for h in range(2):
            ps_s = ps_sim.tile([128, 4, Q + 1], F32, tag="sim")
            for k in range(4):
                cs = h * 4 + k
                csl = slice(cs * 128, (cs + 1) * 128)
                nc.tensor.matmul(
                    ps_s[:, k, :], ctxT[:, 0, csl], rhs_ext[:, 0, :],
                    start=(k == 0), stop=False,
                )
                nc.tensor.matmul(
                    ps_s[:, k, :], ctxT[:, 1, csl], rhs_ext[:, 1, :],
                    start=False, stop=False,
                )
            # qw broadcast rows last: the ones_b stationary is loaded once
            # for all 8 of these matmuls instead of being re-loaded between
            # ctx matmuls
            for k in range(4):
                nc.tensor.matmul(
                    ps_s[:, k, :], ones_b[:], qw_hi[:], start=False, stop=False
                )
            for k in range(4):
                nc.tensor.matmul(
                    ps_s[:, k, :], ones_b[:], qw_lo[:], start=False, stop=(k == 3)
                )
            hl = slice(h * 4, (h + 1) * 4)
            nc.vector.reduce_max(
                out=negrow[:, hl], in_=ps_s[:, :, 0:Q], axis=AX, negate=True
            )
            nc.scalar.copy(
                ctxw_c[:, hl], ps_s[:, :, Q : Q + 1].rearrange("p a b -> p (a b)")
            )
            # max-subtract on DVE (cheap per-chunk scalar add); bf16 is fine
            # post-subtraction (error scales with magnitude, only near-0
            # logits matter)
            for k in range(4):
                cs = h * 4 + k
                nc.vector.tensor_scalar_add(
                    praw[:, cs, :], ps_s[:, k, 0:Q], negrow[:, cs : cs + 1]
                )
        # one big exp (no per-chunk bias / accumulator reads) + one reduction
        # instead of 8 serial ACT activations: frees ACT for evictions
        nc.scalar.activation(out=p[:], in_=praw[:], func=ACTF.Exp, scale=1.0)
        nc.vector.reduce_sum(out=zrows[:], in_=p[:], axis=AX)
        nc.vector.reciprocal(rz[:], zrows[:])
        for cs in range(8):
            nc.vector.tensor_scalar_mul(p[:, cs, :], p[:, cs, :], rz[:, cs : cs + 1])

        # ---- qW2 = question @ W2^T (+ bias: softmax rows sum to 1, so
        # adding b here adds exactly b to the output) ----
        qT16 = work.tile([128, 2, Q], BF16, tag="qT16")
        for ec in range(2):
            nc.vector.tensor_copy(qT16[:, ec, :], qT[:, ec, :])
        qw2 = work.tile([64, FE], BF16, tag="qw2")
        # ec-outer so each qT16 stationary is loaded once for both fh halves
        ps_qs = [
            ps_msc.tile([64, 512], F32, tag="msc", name=f"ps_q{fh}")
            for fh in range(2)
        ]
        for ec in range(2):
            for fh in range(2):
                fhl = slice(fh * 512, (fh + 1) * 512)
                nc.tensor.matmul(
                    ps_qs[fh][:], qT16[:, ec, :], wt[:, ec + 2, fhl],
                    start=(ec == 0), stop=(ec == 1),
                )
        for fh in range(2):
            fhl = slice(fh * 512, (fh + 1) * 512)
            if bias_zero:
                nc.scalar.copy(qw2[:, fhl], ps_qs[fh][:])
            else:
                nc.vector.tensor_add(qw2[:, fhl], ps_qs[fh][:], b_b64[:, fhl])

        return dict(
            b=b, ctxT=ctxT, ctxT16=ctxT16, q16=q16, ctxn16=ctxn16,
            mask_c=mask_c, negrow=negrow, ctxw_c=ctxw_c, p=p, qw2=qw2,
        )

    def _attn_B(st):
        """PE ops that wait on the softmax chain: transposes, c2q, q2c."""
        ctxT, ctxT16, q16, ctxn16 = st["ctxT"], st["ctxT16"], st["q16"], st["ctxn16"]
        negrow, ctxw_c, p = st["negrow"], st["ctxw_c"], st["p"]

        # ---- P^T via PE transposes (4 chunks per psum tile) ----
        pnt = work.tile([64, C], BF16, tag="pnt")  # P_norm^T
        for h in range(2):
            ps_t = ps_msc.tile([64, 4, 128], BF16, tag="msc")
            for k in range(4):
                cs = h * 4 + k
                nc.tensor.transpose(ps_t[:, k, :], p[:, cs, :], ident16[:])
            nc.vector.tensor_copy(
                pnt[:, h * 512 : (h + 1) * 512],
                ps_t[:].rearrange("p a b -> p (a b)"),
            )

        # ---- c2q attention (only needed for block3 = ctx * c2q) ----
        att3 = work.tile([128, 2, C], BF16, tag="att3")  # (ctx*c2q)^T
        for ec in range(2):
            for ch in range(2):
                chl = slice(ch * 512, (ch + 1) * 512)
                ps_c2q = ps_msc.tile([128, 512], F32, tag="msc")
                nc.tensor.matmul(
                    ps_c2q[:],
                    q16[:, ec * 128 : (ec + 1) * 128],
                    pnt[:, chl],
                    start=True,
                    stop=True,
                )
                nc.vector.tensor_mul(att3[:, ec, chl], ctxT[:, ec, chl], ps_c2q[:])

        # ---- q2c attention + attended block 4 (feeds wtm, needed late) ----
        rowtrue = work.tile([128, 8], F32, tag="rowtrue")
        nc.vector.tensor_sub(rowtrue[:], ctxw_c[:], negrow[:])
        colmax = small.tile([128, 1], F32, tag="colmax")
        nc.vector.reduce_max(out=colmax[:], in_=rowtrue[:], axis=AX)
        ps_t1 = ps_msc.tile([1, 128], F32, tag="msc")
        nc.tensor.transpose(ps_t1[:], colmax[:], ident[:])
        tmax = small.tile([1, 128], F32, tag="tmax")
        nc.scalar.copy(tmax[:], ps_t1[:])
        gneg = small.tile([1, 1], F32, tag="gneg")  # -gmax
        nc.vector.reduce_max(out=gneg[:], in_=tmax[:], axis=AX, negate=True)
        ps_gb = ps_msc.tile([128, 1], F32, tag="msc")
        nc.tensor.matmul(ps_gb[:], ones_f[:], gneg[:], start=True, stop=True)
        gneg_col = small.tile([128, 1], F32, tag="gnegc")
        nc.scalar.copy(gneg_col[:], ps_gb[:])
        e_t = work.tile([128, 8], F32, tag="e_t")  # exp(rowtrue - gmax)
        zrow2 = small.tile([128, 1], F32, tag="zrow2")
        nc.scalar.activation(
            out=e_t[:],
            in_=rowtrue[:],
            func=ACTF.Exp,
            bias=gneg_col[:],
            scale=1.0,
            accum_out=zrow2[:],
        )
        e16 = work.tile([128, 8], BF16, tag="e16")
        nc.vector.tensor_copy(e16[:], e_t[:])
        ps_z = ps_msc.tile([1, 1], F32, tag="msc")
        nc.tensor.matmul(ps_z[:], zrow2[:], ones_col[:], start=True, stop=True)
        z_s = small.tile([1, 1], F32, tag="z_s")
        nc.scalar.copy(z_s[:], ps_z[:])
        rz1 = small.tile([1, 1], F32, tag="rz1")
        nc.vector.reciprocal(rz1[:], z_s[:])
        ps_q2c = ps_msc.tile([1, E], F32, tag="msc")
        for j in range(8):
            nc.tensor.matmul(
                ps_q2c[:],
                e16[:, j : j + 1],
                ctxn16[:, j, :],
                start=(j == 0),
                stop=(j == 7),
            )
        # 1/Z folded into the eviction copy
        q2c_row = small.tile([1, E], F32, tag="q2c_row")
        nc.scalar.activation(
            out=q2c_row[:], in_=ps_q2c[:], func=ACTF.Copy, scale=rz1[:]
        )
        # block4 never materializes: (ctx . q2c) @ W4^T == ctx @ (q2c . W4^T),
        # so fold q2c into merged weights for the ctx block instead.
        wtm = work.tile([128, 2, FE], BF16, tag="wtm")  # W1^T + q2c . W4^T
        for ec in range(2):
            ps_qc = ps_msc.tile([128, 1], F32, tag="msc")
            nc.tensor.matmul(
                ps_qc[:],
                q2c_row[0:1, ec * 128 : (ec + 1) * 128],
                ones_f[0:1, 0:1],
                start=True,
                stop=True,
            )
            q2c_col = small.tile([128, 1], F32, tag="q2c_col")
            nc.scalar.copy(q2c_col[:], ps_qc[:])
            nc.vector.tensor_scalar_mul(wtm[:, ec, :], wt[:, ec + 6, :], q2c_col[:])
            nc.vector.tensor_add(wtm[:, ec, :], wtm[:, ec, :], wt[:, ec, :])

        st["att3"], st["wtm"], st["pnt"] = att3, wtm, pnt

        if probe == "attn":
            b = st["b"]
            sink = outp.tile([128, 32], BF16, tag="sink")
            nc.vector.tensor_copy(sink[:, 0:8], st["qw2"][0:64, 0:8])
            nc.vector.tensor_copy(sink[:, 8:16], att3[:, 0, 0:8])
            nc.vector.tensor_copy(sink[:, 16:24], wtm[:, 0, 0:8])
            nc.vector.tensor_copy(sink[:, 24:32], ctxT16[:, 0, 0:8])
            nc.sync.dma_start(dram["out"][b, 0:128, 0:32], sink[:])
            st["skip_final"] = True

    def _final_half(st, half):
        # ---- final matmul: out = (attended @ W^T + b) * mask ----
        if st.get("skip_final"):
            return
        b, ctxT16 = st["b"], st["ctxT16"]
        att3, wtm, mask_c = st["att3"], st["wtm"], st["mask_c"]
        pnt, qw2 = st["pnt"], st["qw2"]
        for cs in range(half * 4, half * 4 + 4):
            csl = slice(cs * 128, (cs + 1) * 128)
            out_s = outp.tile([128, FE], BF16, tag="out")
            # one [128, 2, 512] psum pair (2 banks): each stationary operand
            # (attended^T chunk) is loaded once and used for both output
            # halves, halving LDWEIGHTS traffic.
            # merged ctx block (K=256), P-hat block (K=64), ctx*c2q (K=256)
            ps_o = ps_fin.tile([128, 2, 512], F32, tag="fin")
            blocks = [
                (pnt[:, csl], qw2[:, :]),
                (att3[:, 0, csl], wt[:, 4, :]),
                (att3[:, 1, csl], wt[:, 5, :]),
                (ctxT16[:, 0, csl], wtm[:, 0, :]),
                (ctxT16[:, 1, csl], wtm[:, 1, :]),
            ]
            nblk = len(blocks)
            for i, (lhsT, rhs) in enumerate(blocks):
                for fh in range(2):
                    fhl = slice(fh * 512, (fh + 1) * 512)
                    nc.tensor.matmul(
                        ps_o[:, fh, :], lhsT, rhs[:, fhl],
                        start=(i == 0), stop=(i == nblk - 1),
                    )
            for fh in range(2):
                fhl = slice(fh * 512, (fh + 1) * 512)
                if mask_ones:
                    # alternate evictions between ACT and DVE
                    if fh == 0:
                        nc.scalar.copy(out_s[:, fhl], ps_o[:, fh, :])
                    else:
                        nc.vector.tensor_copy(out_s[:, fhl], ps_o[:, fh, :])
                else:
                    nc.scalar.activation(
                        out=out_s[:, fhl],
                        in_=ps_o[:, fh, :],
                        func=ACTF.Copy,
                        scale=mask_c[:, cs : cs + 1],
                    )
            # one merged output DMA per c-tile, on the ACT HWDGE queue (the
            # inputs use the SP queue; splitting halves descriptor-gen serial
            # time)
            nc.scalar.dma_start(dram["out"][b, csl, :], out_s[:])

    def _all_batches(prev=None):
        # software pipeline with the final matmul split in halves around the
        # softmax-dependent PE ops of the NEXT batch, so the in-order PE
        # stream never waits on the softmax chain:
        #   simMMs(b+1) | final(b).half0 | transposes/c2q/q2c(b+1) |
        #   final(b).half1
        # `prev` carries the pipeline across unrolled loop bodies.
        for b in range(BL):
            st = _attn_A(b)
            if st is None:
                continue
            if prev is None:
                _attn_B(st)
            else:
                _final_half(prev, 0)
                _attn_B(st)
                _final_half(prev, 1)
            prev = st
        return prev

    def _finish(prev):
        if prev is not None:
            _final_half(prev, 0)
            _final_half(prev, 1)

    if loop > 1:
        # unroll the hardware loop: For_i ends each iteration with an
        # all-engine barrier (semaphore reset), so emitting U bodies per
        # iteration amortizes the pipeline drain/ramp 1/U
        U = 8 if loop % 8 == 0 else (4 if loop % 4 == 0 else 1)
        with tc.For_i(
            0,
            loop // U,
            1,
            hint_engines=(
                mybir.EngineType.PE,
                mybir.EngineType.DVE,
                mybir.EngineType.Activation,
                mybir.EngineType.SP,
                mybir.EngineType.Pool,
            ),
        ):
            prev = None
            for _ in range(U):
                prev = _all_batches(prev)
            _finish(prev)
    else:
        _finish(_all_batches())
    if "stub" in dram:
        nc.sync.dma_start(dram["stub"][:], ones_f[0:1, 0:8])


_NC_CACHE = {}


def _get_nc(loop=1, bias_zero=False, mask_ones=False, probe=None):
    key = (loop, bias_zero, mask_ones, probe)
    if key not in _NC_CACHE:
        nc = bass.Bass("TRN2", target_bir_lowering=False, debug=False,
                       num_devices=NCORES)
        dram = {
            "ctxT": nc.dram_tensor("ctxT", [BL, E, C], F32, kind="ExternalInput").ap(),
            "ctxn16": nc.dram_tensor(
                "ctxn16", [BL, C, E], BF16, kind="ExternalInput"
            ).ap(),
            "qT": nc.dram_tensor("qT", [BL, E, Q], F32, kind="ExternalInput").ap(),
            "q16": nc.dram_tensor("q16", [BL, Q, E], BF16, kind="ExternalInput").ap(),
            "mask": nc.dram_tensor("mask", [BL, C], F32, kind="ExternalInput").ap(),
            "wt16": nc.dram_tensor("wt16", [FE, FE], BF16, kind="ExternalInput").ap(),
            "b16": nc.dram_tensor("b16", [FE], BF16, kind="ExternalInput").ap(),
            "vecs": nc.dram_tensor("vecs", [E, 4], F32, kind="ExternalInput").ap(),
        }
        if loop > 1:
            # timing variant: keep the big output on-device, return a stub
            dram["out"] = nc.dram_tensor("out_int", [BL, C, FE], BF16).ap()
            dram["stub"] = nc.dram_tensor(
                "out", [1, 8], F32, kind="ExternalOutput"
            ).ap()
        else:
            dram["out"] = nc.dram_tensor(
                "out", [BL, C, FE], BF16, kind="ExternalOutput"
            ).ap()
        from contextlib import ExitStack

        with tile.TileContext(nc) as tc, ExitStack() as es:
            _emit(nc, tc, dram, es, loop=loop, bias_zero=bias_zero,
                  mask_ones=mask_ones, probe=probe)
        _split_multi_waits(nc)
        _NC_CACHE[key] = nc
    return _NC_CACHE[key]


def _prep_inputs(context, question, context_mask, w_question, w_context, w_multiple,
                 final_W, final_b):
    """Host-side layout prep + sharding. Returns per-core input maps."""
    bf16 = ml_dtypes.bfloat16
    context = np.asarray(context, np.float32)
    question = np.asarray(question, np.float32)
    ctxT = np.ascontiguousarray(context.transpose(0, 2, 1))
    ctx16 = context.astype(bf16)
    qT = np.ascontiguousarray(question.transpose(0, 2, 1))
    q16 = question.astype(bf16)
    wt16 = np.ascontiguousarray(np.asarray(final_W, np.float32).T).astype(bf16)
    b16 = np.asarray(final_b, np.float32).astype(bf16)
    vecs = np.stack(
        [
            np.asarray(w_question, np.float32),
            np.asarray(w_context, np.float32),
            np.asarray(w_multiple, np.float32),
            np.zeros(E, np.float32),
        ],
        axis=1,
    )
    mask = np.asarray(context_mask, np.float32)
    in_maps = []
    for i in range(NCORES):
        s = slice(i * BL, (i + 1) * BL)
        in_maps.append(
            {
                "ctxT": ctxT[s],
                "ctxn16": ctx16[s],
                "qT": qT[s],
                "q16": q16[s],
                "mask": mask[s],
                "wt16": wt16,
                "b16": b16,
                "vecs": vecs,
            }
        )
    return in_maps


def kernel(context, question, context_mask, w_question, w_context, w_multiple,
           final_W, final_b, _loop=1, _probe=None, **run_kwargs):
    bias_zero = not np.any(np.asarray(final_b))
    mask_ones = bool(np.all(np.asarray(context_mask) == 1.0))
    nc = _get_nc(loop=_loop, bias_zero=bias_zero, mask_ones=mask_ones, probe=_probe)
    in_maps = _prep_inputs(
        context, question, context_mask, w_question, w_context, w_multiple,
        final_W, final_b,
    )
    res = run_bass_kernel_spmd(nc, in_maps, core_ids=list(range(NCORES)), **run_kwargs)
    if _loop > 1:
        return res
    out = np.empty((B, C, FE), np.float32)
    for i in range(NCORES):
        out[i * BL : (i + 1) * BL] = res.results[i]["out"].astype(np.float32)
    if run_kwargs:
        kernel.last_results = res
    return out

